# revision 2
# baseline (speedup 1.0000x reference)
"""Mamba mixer Bass kernel for 8 Trainium2 NeuronCores — v3.1.

Tensor-parallel over intermediate_size (4096 -> 512 channels per core).
Engine-rebalanced redesign vs v2 (which was 95% DVE-bound at 1.46ms):
  - depthwise conv on PE (diagonal-matmul PSUM accumulation, silu+bias
    fused into the Act evacuation).
  - scan slot split: DVE keeps scans + all dBu muls + dt0's hc/yadd;
    Pool takes hc muls for dt1-3 and dt1's yadd; dt2/dt3 yadds ride the
    SWDGE accumulate-DMA ring (2 per slot keeps the ring un-contended).
  - AllReduce chunked 4x ([160,512] per chunk tensor) so the head
    pipeline overlaps in_proj with the collective.
  - out_proj PSUM in [128,1024] half tiles, double buffered.
  - out_part in bf16 (halves output DMA; host gather sums in f64).
"""

import numpy as np

import concourse.bass as bass
import concourse.mybir as mybir
import concourse.tile as tile
from concourse.bass_utils import run_bass_kernel_spmd

F32 = mybir.dt.float32
BF16 = mybir.dt.bfloat16
AF = mybir.ActivationFunctionType
OP = mybir.AluOpType

N_CORES = 8

CFG_FULL = dict(
    H=2048,      # hidden size
    IL=512,      # local intermediate channels (4096 / 8)
    N=16,        # ssm state size
    R=128,       # dt rank
    B=2,         # batch
    L=2048,      # sequence length
    # per-dt engine for hc mul and yadd:
    HC_ENG=("vector", "vector", "vector", "vector"),
    YADD_ENG=("vector", "vector", "vector", "vector"),
)


def _split_sync_waits(nc, maxw=1):
    """walrus in this container accepts a single sem-wait per instruction;
    move extra waits onto preceding same-engine drains."""
    cnt = 0
    for bb in nc.main_func.blocks:
        insts = bb.instructions
        i = 0
        while i < len(insts):
            ins = insts[i]
            si = getattr(ins, "sync_info", None)
            waits = list(si.on_wait) if si is not None and si.on_wait else []
            if len(waits) > maxw:
                extra, keep = waits[:-maxw], waits[-maxw:]
                si.on_wait = keep
                pre = []
                for j in range(0, len(extra), maxw):
                    nop = mybir.InstDrain(
                        name=f"{ins.name}-wsplit-{j}", engine=ins.engine)
                    nop.sync_info = mybir.SyncInfo(
                        on_wait=extra[j:j + maxw], on_update=[])
                    pre.append(nop)
                insts[i:i] = pre
                i += len(pre)
                cnt += len(pre)
            i += 1
    return cnt


def _dedup_ldweights(nc):
    """Consecutive PE matmuls with an identical stationary operand skip the
    weight reload (the PE array retains weights across other instructions)."""
    def sig(ap):
        try:
            return str(ap)
        except Exception:
            return None
    cnt = 0
    for bb in nc.main_func.blocks:
        last = None
        for ins in bb.instructions:
            if not isinstance(ins, mybir.InstMatmult):
                continue
            if ins.is_transpose:
                last = None
                continue
            key = (sig(ins.ins[1]) if len(ins.ins) > 1 else None,
                   str(ins.perf_mode), str(ins.tile_position))
            if key[0] is not None and key == last:
                ins.ldweights = False
                cnt += 1
            else:
                last = key
    return cnt


def build_program(cfg):
    H, IL, N, R, B, L = cfg["H"], cfg["IL"], cfg["N"], cfg["R"], cfg["B"], cfg["L"]
    KH = H // 128          # k-tiles of the hidden contraction (16)
    MD = IL // 128         # d-tiles of local channels (4)
    CH = 512               # matmul moving chunk
    NCH = L // CH          # chunks per batch sequence (4)
    KCONV = 4
    HC_ENG = cfg["HC_ENG"]
    YADD_ENG = cfg["YADD_ENG"]
    RN = R + 2 * N

    nc = bass.Bass()

    hsT = nc.declare_dram_parameter("hsT", [H, B * L], BF16, isOutput=False)
    winT = nc.declare_dram_parameter("winT", [H, 2 * IL], BF16, isOutput=False)
    convd = nc.declare_dram_parameter("convd", [128, MD * KCONV * 128], BF16,
                                      isOutput=False)
    convb = nc.declare_dram_parameter("convb", [IL, 1], F32, isOutput=False)
    xwT = nc.declare_dram_parameter("xwT", [IL, RN], BF16, isOutput=False)
    dtwT = nc.declare_dram_parameter("dtwT", [R, IL], BF16, isOutput=False)
    dtb = nc.declare_dram_parameter("dtb", [IL, 1], F32, isOutput=False)
    Amat = nc.declare_dram_parameter("Amat", [IL, N], F32, isOutput=False)
    Dp = nc.declare_dram_parameter("Dp", [IL, 1], F32, isOutput=False)
    woT = nc.declare_dram_parameter("woT", [IL, H], BF16, isOutput=False)
    ident = nc.declare_dram_parameter("ident", [128, 128], BF16, isOutput=False)
    out_part = nc.declare_dram_parameter("out_part", [H, B * L], BF16,
                                         isOutput=True)
    dbg = {}
    if cfg.get("DEBUG"):
        for nm in ("dbg_x", "dbg_u", "dbg_dl", "dbg_du", "dbg_y"):
            dbg[nm] = nc.declare_dram_parameter(nm, [IL, L], BF16,
                                                isOutput=True)

    ssm_p = [nc.dram_tensor(f"ssm_p{b}", [RN, L], BF16) for b in range(B)]
    ssm_f = [nc.dram_tensor(f"ssm_f{b}", [RN, L], BF16, addr_space="Shared")
             for b in range(B)]
    sgd = nc.dram_tensor("sgd", [IL, B * L], BF16)

    with tile.TileContext(nc) as tc:
        with tc.tile_pool(name="const", bufs=1) as cp, \
             tc.tile_pool(name="ab", bufs=1) as ab, \
             tc.tile_pool(name="upool", bufs=1) as up, \
             tc.tile_pool(name="dupool", bufs=1) as dup, \
             tc.tile_pool(name="ypool", bufs=1) as yp, \
             tc.tile_pool(name="work", bufs=1) as wk, \
             tc.tile_pool(name="bcast", bufs=4) as bc, \
             tc.tile_pool(name="hstream", bufs=1) as hp, \
             tc.tile_pool(name="stage", bufs=1) as stg, \
             tc.tile_pool(name="win", bufs=1) as wp, \
             tc.tile_pool(name="psY", bufs=1, space="PSUM") as ppY, \
             tc.tile_pool(name="psSmA", bufs=2, space="PSUM") as ppA, \
             tc.tile_pool(name="psO", bufs=1, space="PSUM") as ppO:

            # ---- constants ----
            A_t, cb_t, db_t, D_t, xw_t, dtw_t, cd_t = [], [], [], [], [], [], []
            for dt in range(MD):
                rows = slice(dt * 128, (dt + 1) * 128)
                a = cp.tile([128, N], F32, name=f"A{dt}", tag=f"A{dt}")
                nc.sync.dma_start(a[:], Amat[rows, :])
                A_t.append(a)
                b_ = cp.tile([128, 1], F32, name=f"cb{dt}", tag=f"cb{dt}")
                nc.sync.dma_start(b_[:], convb[rows, :])
                cb_t.append(b_)
                d_ = cp.tile([128, 1], F32, name=f"db{dt}", tag=f"db{dt}")
                nc.sync.dma_start(d_[:], dtb[rows, :])
                db_t.append(d_)
                dd = cp.tile([128, 1], F32, name=f"Dp{dt}", tag=f"Dp{dt}")
                nc.sync.dma_start(dd[:], Dp[rows, :])
                D_t.append(dd)
                xw = cp.tile([128, RN], BF16, name=f"xw{dt}", tag=f"xw{dt}")
                nc.sync.dma_start(xw[:], xwT[rows, :])
                xw_t.append(xw)
                dw = cp.tile([128, 128], BF16, name=f"dtw{dt}", tag=f"dtw{dt}")
                nc.sync.dma_start(dw[:], dtwT[:, rows])
                dtw_t.append(dw)
                cds = []
                for s in range(KCONV):
                    cdt = cp.tile([128, 128], BF16, name=f"cd{dt}_{s}",
                                  tag=f"cd{dt}_{s}")
                    off = (dt * KCONV + s) * 128
                    nc.sync.dma_start(cdt[:], convd[:, off:off + 128])
                    cds.append(cdt)
                cd_t.append(cds)

            I_t = cp.tile([128, 128], BF16, name="ident", tag="ident")
            nc.sync.dma_start(I_t[:], ident[:])

            # ---- lifetime-shared activation tiles ----
            def tagA(dt, nm):          # x0 -> x1 -> dl1
                return ab.tile([128, L], BF16, name=nm, tag=f"tA{dt}")

            def tagB(dt, nm):          # dl0 -> y1
                return ab.tile([128, L], BF16, name=nm, tag=f"tB{dt}")

            u_t = [up.tile([128, L], BF16, name=f"u{dt}", tag=f"u{dt}")
                   for dt in range(MD)]
            du_t = [dup.tile([128, L], BF16, name=f"du{dt}", tag=f"du{dt}")
                    for dt in range(MD)]
            y_t = [yp.tile([128, L], BF16, name=f"y{dt}", tag=f"y{dt}")
                   for dt in range(MD)]

            state = {"x": {}}

            def load_win(half):
                w_cur = []
                for k in range(KH):
                    wt = wp.tile([128, CH], BF16, name=f"wk{k}", tag=f"wk{k}")
                    nc.sync.dma_start(
                        wt[:], winT[k * 128:(k + 1) * 128,
                                    half * CH:(half + 1) * CH])
                    w_cur.append(wt)
                return w_cur

            def in_proj_chunk(b, c, w_cur, half, x_cur):
                boff = b * L
                csl = slice(c * CH, (c + 1) * CH)
                hst = []
                for k in range(KH):
                    ht = hp.tile([128, CH], BF16, name=f"hs{k}", tag=f"hs{k}")
                    nc.sync.dma_start(
                        ht[:], hsT[k * 128:(k + 1) * 128,
                                   boff + csl.start:boff + csl.stop])
                    hst.append(ht)
                for ml in range(MD):
                    msl = slice(ml * 128, (ml + 1) * 128)
                    ps = ppA.tile([128, CH], F32, name="psA", tag="psA")
                    for k in range(KH):
                        nc.tensor.matmul(ps[:], w_cur[k][:, msl], hst[k][:],
                                         start=(k == 0), stop=(k == KH - 1))
                    if half == 0:
                        nc.scalar.copy(x_cur[ml][:, csl], ps[:])
                    else:
                        sgs = stg.tile([128, CH], BF16, name="sgs", tag="sgs",
                                       bufs=1)
                        nc.scalar.activation(sgs[:], ps[:], AF.Silu)
                        nc.scalar.dma_start(
                            sgd[ml * 128:(ml + 1) * 128,
                                boff + csl.start:boff + csl.stop], sgs[:])

            def conv_pe(b, chunks=None, dts=None):
                x_cur = state["x"][b]
                for c in (chunks if chunks is not None else range(NCH)):
                    lo = c * CH
                    for dt in (dts if dts is not None else range(MD)):
                        ps = ppA.tile([128, CH], F32, name="psC", tag="psA")
                        for s in range(KCONV):
                            w_s = cd_t[dt][s]
                            if c == 0:
                                nc.tensor.matmul(ps[:, s:] if s else ps[:],
                                                 w_s[:],
                                                 x_cur[dt][:, 0:CH - s],
                                                 start=(s == 0),
                                                 stop=(s == KCONV - 1),
                                                 skip_group_check=True)
                            else:
                                nc.tensor.matmul(
                                    ps[:], w_s[:],
                                    x_cur[dt][:, lo - s:lo + CH - s],
                                    start=(s == 0), stop=(s == KCONV - 1))
                        nc.scalar.activation(u_t[dt][:, lo:lo + CH], ps[:],
                                             AF.Silu, bias=cb_t[dt][:, 0:1])

            def x_proj(b, chunks=None):
                for c in (chunks if chunks is not None else range(NCH)):
                    csl = slice(c * CH, (c + 1) * CH)
                    psa = ppA.tile([128, CH], F32, name="psXa", tag="psA")
                    for dt in range(MD):
                        nc.tensor.matmul(psa[:], xw_t[dt][:, 0:R],
                                         u_t[dt][:, csl],
                                         start=(dt == 0), stop=(dt == MD - 1))
                    sta = stg.tile([128, CH], BF16, name="stXa", tag="stXa",
                                   bufs=1)
                    nc.scalar.copy(sta[:], psa[:])
                    nc.scalar.dma_start(ssm_p[b][0:R, csl], sta[:])
                    psb = ppA.tile([128, CH], F32, name="psXb", tag="psA")
                    for dt in range(MD):
                        nc.tensor.matmul(psb[0:2 * N, :], xw_t[dt][:, R:RN],
                                         u_t[dt][:, csl],
                                         start=(dt == 0), stop=(dt == MD - 1))
                    stb = stg.tile([2 * N, CH], BF16, name="stXb", tag="stXb",
                                   bufs=2)
                    nc.scalar.copy(stb[:], psb[0:2 * N, :])
                    nc.scalar.dma_start(ssm_p[b][R:RN, csl], stb[:])

            def all_reduce(b):
                nc.gpsimd.collective_compute(
                    "AllReduce", OP.add,
                    replica_groups=[list(range(N_CORES))],
                    ins=[ssm_p[b][:, :]],
                    outs=[ssm_f[b][:, :]],
                )

            def dt_proj(b, dl_cur, dt):
                """dt_proj -> softplus (batched Exp then batched Ln)."""
                exs = []
                for c in range(NCH):
                    csl = slice(c * CH, (c + 1) * CH)
                    lr = stg.tile([128, CH], BF16, name="lr", tag="lr", bufs=1)
                    nc.scalar.dma_start(lr[:], ssm_f[b][0:R, csl])
                    ps = ppA.tile([128, CH], F32, name="psD", tag="psA")
                    nc.tensor.matmul(ps[:], dtw_t[dt][:], lr[:],
                                     start=True, stop=True)
                    ex = stg.tile([128, CH], BF16, name="ex", tag=f"ex{c % 2}",
                                  bufs=1)
                    nc.scalar.activation(ex[:], ps[:], AF.Exp,
                                         bias=db_t[dt][:, 0:1])
                    exs.append((ex, csl))
                for ex, csl in exs:
                    nc.scalar.activation(dl_cur[dt][:, csl], ex[:], AF.Ln,
                                         bias=1.0)

            def prep_du(b, dl_cur, eng="vector"):
                for dt in range(MD):
                    getattr(nc, eng).tensor_mul(du_t[dt][:], dl_cur[dt][:],
                                                u_t[dt][:])

            def bcast_n(b, n):
                bt = bc.tile([128, L], BF16, name="Bt", tag="Bt", bufs=6)
                nc.sync.dma_start(
                    bt[:], ssm_f[b][R + n:R + n + 1, :].to_broadcast((128, L)))
                ct = bc.tile([128, L], BF16, name="Ct", tag="Ct")
                nc.sync.dma_start(
                    ct[:],
                    ssm_f[b][R + N + n:R + N + n + 1, :].to_broadcast((128, L)))
                return bt, ct

            def gating(b, y_cur):
                boff = b * L
                for dt in range(MD):
                    sg = wk.tile([128, L], BF16, name="sg", tag="hc", bufs=2)
                    nc.scalar.dma_start(
                        sg[:], sgd[dt * 128:(dt + 1) * 128, boff:boff + L])
                    nc.vector.tensor_mul(y_cur[dt][:], y_cur[dt][:], sg[:])

            def load_wo(mhalf):
                wo_k = []
                for kk in range(MD):
                    wt = wp.tile([128, H // 2], BF16, name=f"wo{kk}",
                                 tag=f"wk{kk}")
                    nc.sync.dma_start(
                        wt[:], woT[kk * 128:(kk + 1) * 128,
                                   mhalf * (H // 2):(mhalf + 1) * (H // 2)])
                    wo_k.append(wt)
                return wo_k

            def out_proj_m(b, m, wo_k, yo_cur, alt=False):
                boff = b * L
                ml = m % (KH // 2)
                wsl = slice(ml * 128, (ml + 1) * 128)
                osl = slice(m * 128, (m + 1) * 128)
                for hh in range(2):                   # L halves, 2-bank psum
                    hsl = slice(hh * (L // 2), (hh + 1) * (L // 2))
                    if alt and (m + hh) % 2:
                        ps = ppY.tile([128, L // 2], F32, name="psOy",
                                      tag="psY")
                    else:
                        ps = ppO.tile([128, L // 2], F32, name="psO",
                                      tag="psO")
                    for kk in range(MD):
                        for c in range(2):
                            csl = slice(hh * (L // 2) + c * CH,
                                        hh * (L // 2) + (c + 1) * CH)
                            psl = slice(c * CH, (c + 1) * CH)
                            nc.tensor.matmul(ps[:, psl], wo_k[kk][:, wsl],
                                             yo_cur[kk][:, csl],
                                             start=(kk == 0),
                                             stop=(kk == MD - 1))
                    so = stg.tile([128, L // 2], BF16, name="stO", tag="stO",
                                  bufs=1)
                    nc.scalar.copy(so[:], ps[:])
                    nc.scalar.dma_start(
                        out_part[osl, boff + hsl.start:boff + hsl.stop], so[:])

            # ---- scan group: one (n-block, dt) — 4 scans + PE y-accum ----
            def issue_dA_grp(dt, dl_cur, nb):
                tiles = []
                for j in range(4):
                    n = nb * 4 + j
                    dA = wk.tile([128, L], BF16, name="dA", tag="dA", bufs=4)
                    nc.scalar.activation(dA[:], dl_cur[dt][:], AF.Exp,
                                         scale=A_t[dt][:, n:n + 1])
                    tiles.append(dA)
                return tiles

            def scan_group(y_cur, dt, bts, cts, dAs):
                """4 scans of one dt over an n-block; y accumulated on PE
                into a [128,L] psum via identity matmuls."""
                psY = ppY.tile([128, L], F32, name="psY", tag="psY")
                for c in range(NCH):
                    csl = slice(c * CH, (c + 1) * CH)
                    nc.tensor.matmul(psY[:, csl], I_t[:], y_cur[dt][:, csl],
                                     start=True, stop=False,
                                     skip_group_check=True)
                for j in range(4):
                    dbu = wk.tile([128, L], BF16, name="dBu", tag="dBu",
                                  bufs=1)
                    nc.vector.tensor_mul(dbu[:], du_t[dt][:], bts[j][:])
                    h = wk.tile([128, L], BF16, name="h", tag="h", bufs=1)
                    nc.vector.tensor_tensor_scan(h[:], dAs[j][:], dbu[:],
                                                 0.0, op0=OP.mult, op1=OP.add)
                    hc = wk.tile([128, L], BF16, name="hc", tag="hc", bufs=2)
                    nc.vector.tensor_mul(hc[:], h[:], cts[j][:])
                    for c in range(NCH):
                        csl = slice(c * CH, (c + 1) * CH)
                        nc.tensor.matmul(psY[:, csl], I_t[:], hc[:, csl],
                                         start=False,
                                         stop=(j == 3 and c == NCH - 1),
                                         skip_group_check=True)
                nc.scalar.copy(y_cur[dt][:], psY[:])

            def issue_block0(b):
                bts, cts = [], []
                for j in range(4):
                    t1, t2 = bcast_n(b, j)
                    bts.append(t1)
                    cts.append(t2)
                return bts, cts

            # ================= schedule =================
            # ---- head: batch 0 x-pass, chunk-pipelined, chunked AR ----
            w_x = load_win(0)
            x0 = [tagA(dt, f"x0_{dt}") for dt in range(MD)]
            state["x"][0] = x0
            for c in range(NCH):
                in_proj_chunk(0, c, w_x, 0, x0)
                conv_pe(0, chunks=[c])
                x_proj(0, [c])
            all_reduce(0)
            pre0_b0 = issue_block0(0)
            dl0 = [tagB(dt, f"dl0_{dt}") for dt in range(MD)]

            def prep_b0_dt(dt):
                dt_proj(0, dl0, dt)
                nc.vector.tensor_mul(du_t[dt][:], dl0[dt][:], u_t[dt][:])
                nc.scalar.activation(y_t[dt][:], u_t[dt][:], AF.Copy,
                                     scale=D_t[dt][:, 0:1])

            prep_b0_dt(0)
            if cfg.get("DEBUG"):
                for dt in range(MD):
                    rs = slice(dt * 128, (dt + 1) * 128)
                    nc.sync.dma_start(dbg["dbg_x"][rs, :], x0[dt][:])
                    nc.sync.dma_start(dbg["dbg_u"][rs, :], u_t[dt][:])

            # ---- scan window 0 (batch 0): n-blocks of 4, dt-inner ----
            x1 = [tagA(dt, f"x1_{dt}") for dt in range(MD)]
            state["x"][1] = x1
            dl1 = None
            w_g = None

            def window(b, y_cur, dl_cur, interleave, pre0,
                       tail_prefetch=None):
                bts = [pre0[0], None]
                cts = [pre0[1], None]
                nxt_dA = issue_dA_grp(0, dl_cur, 0)
                for nb in range(4):
                    if nb + 1 < 4:
                        bts[1], cts[1] = [], []
                        for j in range(4):
                            t1, t2 = bcast_n(b, (nb + 1) * 4 + j)
                            bts[1].append(t1)
                            cts[1].append(t2)
                    if nb == 3 and tail_prefetch is not None:
                        tail_prefetch()
                    for dt in range(MD):
                        g = nb * 4 + dt
                        cur_dA = nxt_dA
                        interleave(g)
                        if g + 1 < 16:
                            nxt_dA = issue_dA_grp((dt + 1) % MD, dl_cur,
                                                  nb + (dt + 1) // MD)
                        scan_group(y_cur, dt, bts[0], cts[0], cur_dA)
                    bts[0], cts[0] = bts[1], cts[1]



            def interleave0(g):
                nonlocal dl1, w_g
                if 0 <= g <= 2:                    # deferred b0 prep
                    prep_b0_dt(g + 1)
                if 1 <= g <= 4:                    # in_proj-x (b1)
                    in_proj_chunk(1, g - 1, w_x, 0, x1)
                if 2 <= g <= 5:                    # conv (b1)
                    conv_pe(1, chunks=[g - 2])
                if g == 5:                         # x_proj (b1)
                    x_proj(1)
                if g == 6:
                    all_reduce(1)
                    w_g = load_win(1)
                if 7 <= g <= 10:                   # gate pass (b0)
                    in_proj_chunk(0, g - 7, w_g, 1, None)
                if 11 <= g <= 14:                  # dt_proj (b1)
                    if dl1 is None:
                        dl1 = [tagA(dt, f"dl1_{dt}") for dt in range(MD)]
                    dt_proj(1, dl1, g - 11)

            pre_hold = {}

            def tail_pf():
                pre_hold["b1"] = issue_block0(1)

            window(0, y_t, dl0, interleave0, pre0_b0, tail_pf)

            if cfg.get("DEBUG"):
                for dt in range(MD):
                    rs = slice(dt * 128, (dt + 1) * 128)
                    nc.sync.dma_start(dbg["dbg_dl"][rs, :], dl0[dt][:])
                    nc.sync.dma_start(dbg["dbg_du"][rs, :], du_t[dt][:])
                    nc.sync.dma_start(dbg["dbg_y"][rs, :], y_t[dt][:])

            yo0 = y_t

            # ---- scan window 1 (batch 1) ----
            y1 = [tagB(dt, f"y1_{dt}") for dt in range(MD)]
            prep_du(1, dl1, eng="vector")
            for dt in range(MD):
                nc.scalar.activation(y1[dt][:], u_t[dt][:], AF.Copy,
                                     scale=D_t[dt][:, 0:1])

            wo_state = {"wo": None}

            def interleave1(g):
                if g == 0:
                    gating(0, y_t)  # y0 *= silu(gate0), feeds out_proj at g>=5
                if 1 <= g <= 4:                    # gate pass (b1)
                    in_proj_chunk(1, g - 1, w_g, 1, None)
                if g == 4:
                    wo_state["wo"] = load_wo(0)
                if 5 <= g <= 12:                   # out_proj(0) m=0..15
                    m = 2 * (g - 5)
                    if m == KH // 2:
                        wo_state["wo"] = load_wo(1)
                    out_proj_m(0, m, wo_state["wo"], yo0)
                    out_proj_m(0, m + 1, wo_state["wo"], yo0)

            window(1, y1, dl1, interleave1, pre_hold["b1"])

            # ---- tail: gating(b1) + out_proj(1) ----
            gating(1, y1)
            wo_k = load_wo(0)
            for m in range(KH):
                if m == KH // 2:
                    wo_k = load_wo(1)
                out_proj_m(1, m, wo_k, y1, alt=True)

    _split_sync_waits(nc)
    return nc


def make_in_maps(cfg, hidden_states, in_proj_w, conv_w, conv_b, x_proj_w,
                 dt_proj_w, dt_proj_b, A_log, D_param, out_proj_w):
    import ml_dtypes
    BF = ml_dtypes.bfloat16
    H, IL, N, R, B, L = cfg["H"], cfg["IL"], cfg["N"], cfg["R"], cfg["B"], cfg["L"]
    MD = IL // 128
    KCONV = 4
    BL = B * L
    I_full = IL * N_CORES
    c = np.ascontiguousarray
    hs = np.asarray(hidden_states, np.float32)
    hsT = c(hs.reshape(BL, H).T.astype(BF))
    A_full = -np.exp(np.asarray(A_log, np.float32))
    in_proj_w = np.asarray(in_proj_w, np.float32)
    conv_w_f = np.asarray(conv_w, np.float32)
    in_maps = []
    for ci in range(N_CORES):
        sl = slice(ci * IL, (ci + 1) * IL)
        gsl = slice(I_full + ci * IL, I_full + (ci + 1) * IL)
        wxT = in_proj_w[sl, :].T
        wgT = in_proj_w[gsl, :].T
        convd = np.zeros((128, MD * KCONV * 128), np.float32)
        for dt in range(MD):
            for s in range(KCONV):
                w = conv_w_f[ci * IL + dt * 128:ci * IL + (dt + 1) * 128, 0,
                             KCONV - 1 - s]
                off = (dt * KCONV + s) * 128
                convd[:, off:off + 128][np.arange(128), np.arange(128)] = w
        in_maps.append({
            "hsT": hsT,
            "ident": np.eye(128, dtype=np.float32).astype(BF),
            "winT": c(np.concatenate([wxT, wgT], axis=1).astype(BF)),
            "convd": c(convd.astype(BF)),
            "convb": c(np.asarray(conv_b, np.float32)[sl].reshape(IL, 1)),
            "xwT": c(np.asarray(x_proj_w, np.float32)[:, sl].T.astype(BF)),
            "dtwT": c(np.asarray(dt_proj_w, np.float32)[sl, :].T.astype(BF)),
            "dtb": c(np.asarray(dt_proj_b, np.float32)[sl].reshape(IL, 1)),
            "Amat": c(A_full[sl, :]),
            "Dp": c(np.asarray(D_param, np.float32)[sl].reshape(IL, 1)),
            "woT": c(np.asarray(out_proj_w, np.float32)[:, sl].T.astype(BF)),
        })
    return in_maps


_PROG_CACHE = {}


def run(cfg, inputs, **run_kwargs):
    key = tuple(sorted((k, str(v)) for k, v in cfg.items()))
    if key not in _PROG_CACHE:
        _PROG_CACHE[key] = build_program(cfg)
    nc = _PROG_CACHE[key]
    in_maps = make_in_maps(cfg, **inputs)
    res = run_bass_kernel_spmd(nc, in_maps, list(range(N_CORES)), **run_kwargs)
    H, B, L = cfg["H"], cfg["B"], cfg["L"]
    out = np.zeros((H, B * L), np.float64)
    for ci in range(N_CORES):
        out += np.asarray(res.results[ci]["out_part"], np.float64)
    full = out.astype(np.float32).T.reshape(B, L, H)
    return full, res


def kernel(**inputs):
    out, _ = run(CFG_FULL, inputs)
    return out


# revision 4
# speedup vs baseline: 1.1640x; 1.1640x over previous
"""Mamba mixer Bass kernel for 8 Trainium2 NeuronCores — v3.1.

Tensor-parallel over intermediate_size (4096 -> 512 channels per core).
Engine-rebalanced redesign vs v2 (which was 95% DVE-bound at 1.46ms):
  - depthwise conv on PE (diagonal-matmul PSUM accumulation, silu+bias
    fused into the Act evacuation).
  - scan slot split: DVE keeps scans + all dBu muls + dt0's hc/yadd;
    Pool takes hc muls for dt1-3 and dt1's yadd; dt2/dt3 yadds ride the
    SWDGE accumulate-DMA ring (2 per slot keeps the ring un-contended).
  - AllReduce chunked 4x ([160,512] per chunk tensor) so the head
    pipeline overlaps in_proj with the collective.
  - out_proj PSUM in [128,1024] half tiles, double buffered.
  - out_part in bf16 (halves output DMA; host gather sums in f64).
"""

import numpy as np

import concourse.bass as bass
import concourse.mybir as mybir
import concourse.tile as tile
from concourse.bass_utils import run_bass_kernel_spmd

F32 = mybir.dt.float32
BF16 = mybir.dt.bfloat16
AF = mybir.ActivationFunctionType
OP = mybir.AluOpType

N_CORES = 8

CFG_FULL = dict(
    H=2048,      # hidden size
    IL=512,      # local intermediate channels (4096 / 8)
    N=16,        # ssm state size
    R=128,       # dt rank
    B=2,         # batch
    L=2048,      # sequence length
    # per-dt engine for hc mul and yadd:
    HC_ENG=("vector", "vector", "vector", "vector"),
    YADD_ENG=("vector", "vector", "vector", "vector"),
)


def _split_sync_waits(nc, maxw=1):
    """walrus in this container accepts a single sem-wait per instruction;
    move extra waits onto preceding same-engine drains."""
    cnt = 0
    for bb in nc.main_func.blocks:
        insts = bb.instructions
        i = 0
        while i < len(insts):
            ins = insts[i]
            si = getattr(ins, "sync_info", None)
            waits = list(si.on_wait) if si is not None and si.on_wait else []
            if len(waits) > maxw:
                extra, keep = waits[:-maxw], waits[-maxw:]
                si.on_wait = keep
                pre = []
                for j in range(0, len(extra), maxw):
                    nop = mybir.InstDrain(
                        name=f"{ins.name}-wsplit-{j}", engine=ins.engine)
                    nop.sync_info = mybir.SyncInfo(
                        on_wait=extra[j:j + maxw], on_update=[])
                    pre.append(nop)
                insts[i:i] = pre
                i += len(pre)
                cnt += len(pre)
            i += 1
    return cnt


def _dedup_ldweights(nc):
    """Consecutive PE matmuls with an identical stationary operand skip the
    weight reload (the PE array retains weights across other instructions)."""
    def sig(ap):
        try:
            return str(ap)
        except Exception:
            return None
    cnt = 0
    for bb in nc.main_func.blocks:
        last = None
        for ins in bb.instructions:
            if not isinstance(ins, mybir.InstMatmult):
                continue
            if ins.is_transpose:
                last = None
                continue
            key = (sig(ins.ins[1]) if len(ins.ins) > 1 else None,
                   str(ins.perf_mode), str(ins.tile_position))
            if key[0] is not None and key == last:
                ins.ldweights = False
                cnt += 1
            else:
                last = key
    return cnt


def build_program(cfg):
    H, IL, N, R, B, L = cfg["H"], cfg["IL"], cfg["N"], cfg["R"], cfg["B"], cfg["L"]
    KH = H // 128          # k-tiles of the hidden contraction (16)
    MD = IL // 128         # d-tiles of local channels (4)
    CH = 512               # matmul moving chunk
    NCH = L // CH          # chunks per batch sequence (4)
    KCONV = 4
    HC_ENG = cfg["HC_ENG"]
    YADD_ENG = cfg["YADD_ENG"]
    RN = R + 2 * N

    nc = bass.Bass()

    hsT = nc.declare_dram_parameter("hsT", [H, B * L], BF16, isOutput=False)
    winT = nc.declare_dram_parameter("winT", [H, 2 * IL], BF16, isOutput=False)
    convd = nc.declare_dram_parameter("convd", [128, MD * KCONV * 128], BF16,
                                      isOutput=False)
    convb = nc.declare_dram_parameter("convb", [IL, 1], F32, isOutput=False)
    xwT = nc.declare_dram_parameter("xwT", [IL, RN], BF16, isOutput=False)
    dtwT = nc.declare_dram_parameter("dtwT", [R, IL], BF16, isOutput=False)
    dtb = nc.declare_dram_parameter("dtb", [IL, 1], F32, isOutput=False)
    Amat = nc.declare_dram_parameter("Amat", [IL, N], F32, isOutput=False)
    Dp = nc.declare_dram_parameter("Dp", [IL, 1], F32, isOutput=False)
    woT = nc.declare_dram_parameter("woT", [IL, H], BF16, isOutput=False)
    ident = nc.declare_dram_parameter("ident", [128, 128], BF16, isOutput=False)
    out_part = nc.declare_dram_parameter("out_part", [H, B * L], BF16,
                                         isOutput=True)
    dbg = {}
    if cfg.get("DEBUG"):
        for nm in ("dbg_x", "dbg_u", "dbg_dl", "dbg_du", "dbg_y"):
            dbg[nm] = nc.declare_dram_parameter(nm, [IL, L], BF16,
                                                isOutput=True)

    ssm_p = [nc.dram_tensor(f"ssm_p{b}", [RN, L], BF16) for b in range(B)]
    ssm_f = [nc.dram_tensor(f"ssm_f{b}", [RN, L], BF16, addr_space="Shared")
             for b in range(B)]
    sgd = nc.dram_tensor("sgd", [IL, B * L], BF16)

    with tile.TileContext(nc) as tc:
        with tc.tile_pool(name="const", bufs=1) as cp, \
             tc.tile_pool(name="ab", bufs=1) as ab, \
             tc.tile_pool(name="upool", bufs=1) as up, \
             tc.tile_pool(name="dupool", bufs=1) as dup, \
             tc.tile_pool(name="ypool", bufs=1) as yp, \
             tc.tile_pool(name="work", bufs=1) as wk, \
             tc.tile_pool(name="bcast", bufs=4) as bc, \
             tc.tile_pool(name="hstream", bufs=1) as hp, \
             tc.tile_pool(name="stage", bufs=1) as stg, \
             tc.tile_pool(name="win", bufs=1) as wp, \
             tc.tile_pool(name="psY", bufs=1, space="PSUM") as ppY, \
             tc.tile_pool(name="psSmA", bufs=2, space="PSUM") as ppA, \
             tc.tile_pool(name="psO", bufs=1, space="PSUM") as ppO:

            # ---- constants ----
            A_t, cb_t, db_t, D_t, xw_t, dtw_t, cd_t = [], [], [], [], [], [], []
            for dt in range(MD):
                rows = slice(dt * 128, (dt + 1) * 128)
                a = cp.tile([128, N], F32, name=f"A{dt}", tag=f"A{dt}")
                nc.sync.dma_start(a[:], Amat[rows, :])
                A_t.append(a)
                b_ = cp.tile([128, 1], F32, name=f"cb{dt}", tag=f"cb{dt}")
                nc.sync.dma_start(b_[:], convb[rows, :])
                cb_t.append(b_)
                d_ = cp.tile([128, 1], F32, name=f"db{dt}", tag=f"db{dt}")
                nc.sync.dma_start(d_[:], dtb[rows, :])
                db_t.append(d_)
                dd = cp.tile([128, 1], F32, name=f"Dp{dt}", tag=f"Dp{dt}")
                nc.sync.dma_start(dd[:], Dp[rows, :])
                D_t.append(dd)
                xw = cp.tile([128, RN], BF16, name=f"xw{dt}", tag=f"xw{dt}")
                nc.sync.dma_start(xw[:], xwT[rows, :])
                xw_t.append(xw)
                dw = cp.tile([128, 128], BF16, name=f"dtw{dt}", tag=f"dtw{dt}")
                nc.sync.dma_start(dw[:], dtwT[:, rows])
                dtw_t.append(dw)
                cds = []
                for s in range(KCONV):
                    cdt = cp.tile([128, 128], BF16, name=f"cd{dt}_{s}",
                                  tag=f"cd{dt}_{s}")
                    off = (dt * KCONV + s) * 128
                    nc.sync.dma_start(cdt[:], convd[:, off:off + 128])
                    cds.append(cdt)
                cd_t.append(cds)

            I_t = cp.tile([128, 128], BF16, name="ident", tag="ident")
            nc.sync.dma_start(I_t[:], ident[:])

            # ---- lifetime-shared activation tiles ----
            def tagA(dt, nm):          # x0 -> x1 -> dl1
                return ab.tile([128, L], BF16, name=nm, tag=f"tA{dt}")

            def tagB(dt, nm):          # dl0 -> y1
                return ab.tile([128, L], BF16, name=nm, tag=f"tB{dt}")

            u_t = [up.tile([128, L], BF16, name=f"u{dt}", tag=f"u{dt}")
                   for dt in range(MD)]
            du_t = [dup.tile([128, L], BF16, name=f"du{dt}", tag=f"du{dt}")
                    for dt in range(MD)]
            y_t = [yp.tile([128, L], BF16, name=f"y{dt}", tag=f"y{dt}")
                   for dt in range(MD)]

            state = {"x": {}}

            def load_win(half):
                w_cur = []
                for k in range(KH):
                    wt = wp.tile([128, CH], BF16, name=f"wk{k}", tag=f"wk{k}")
                    nc.sync.dma_start(
                        wt[:], winT[k * 128:(k + 1) * 128,
                                    half * CH:(half + 1) * CH])
                    w_cur.append(wt)
                return w_cur

            def in_proj_chunk(b, c, w_cur, half, x_cur):
                boff = b * L
                csl = slice(c * CH, (c + 1) * CH)
                hst = []
                for k in range(KH):
                    ht = hp.tile([128, CH], BF16, name=f"hs{k}", tag=f"hs{k}")
                    nc.sync.dma_start(
                        ht[:], hsT[k * 128:(k + 1) * 128,
                                   boff + csl.start:boff + csl.stop])
                    hst.append(ht)
                for ml in range(MD):
                    msl = slice(ml * 128, (ml + 1) * 128)
                    ps = ppA.tile([128, CH], F32, name="psA", tag="psA")
                    for k in range(KH):
                        nc.tensor.matmul(ps[:], w_cur[k][:, msl], hst[k][:],
                                         start=(k == 0), stop=(k == KH - 1))
                    if half == 0:
                        nc.scalar.copy(x_cur[ml][:, csl], ps[:])
                    else:
                        sgs = stg.tile([128, CH], BF16, name="sgs", tag="sgs",
                                       bufs=1)
                        nc.scalar.activation(sgs[:], ps[:], AF.Silu)
                        nc.scalar.dma_start(
                            sgd[ml * 128:(ml + 1) * 128,
                                boff + csl.start:boff + csl.stop], sgs[:])

            def conv_pe(b, chunks=None, dts=None):
                x_cur = state["x"][b]
                for c in (chunks if chunks is not None else range(NCH)):
                    lo = c * CH
                    for dt in (dts if dts is not None else range(MD)):
                        ps = ppA.tile([128, CH], F32, name="psC", tag="psA")
                        for s in range(KCONV):
                            w_s = cd_t[dt][s]
                            if c == 0:
                                nc.tensor.matmul(ps[:, s:] if s else ps[:],
                                                 w_s[:],
                                                 x_cur[dt][:, 0:CH - s],
                                                 start=(s == 0),
                                                 stop=(s == KCONV - 1),
                                                 skip_group_check=True)
                            else:
                                nc.tensor.matmul(
                                    ps[:], w_s[:],
                                    x_cur[dt][:, lo - s:lo + CH - s],
                                    start=(s == 0), stop=(s == KCONV - 1))
                        nc.scalar.activation(u_t[dt][:, lo:lo + CH], ps[:],
                                             AF.Silu, bias=cb_t[dt][:, 0:1])

            def x_proj(b, chunks=None):
                for c in (chunks if chunks is not None else range(NCH)):
                    csl = slice(c * CH, (c + 1) * CH)
                    psa = ppA.tile([128, CH], F32, name="psXa", tag="psA")
                    for dt in range(MD):
                        nc.tensor.matmul(psa[:], xw_t[dt][:, 0:R],
                                         u_t[dt][:, csl],
                                         start=(dt == 0), stop=(dt == MD - 1))
                    sta = stg.tile([128, CH], BF16, name="stXa", tag="stXa",
                                   bufs=1)
                    nc.scalar.copy(sta[:], psa[:])
                    nc.scalar.dma_start(ssm_p[b][0:R, csl], sta[:])
                    psb = ppA.tile([128, CH], F32, name="psXb", tag="psA")
                    for dt in range(MD):
                        nc.tensor.matmul(psb[0:2 * N, :], xw_t[dt][:, R:RN],
                                         u_t[dt][:, csl],
                                         start=(dt == 0), stop=(dt == MD - 1))
                    stb = stg.tile([2 * N, CH], BF16, name="stXb", tag="stXb",
                                   bufs=1)
                    nc.scalar.copy(stb[:], psb[0:2 * N, :])
                    nc.scalar.dma_start(ssm_p[b][R:RN, csl], stb[:])

            def all_reduce(b):
                nc.gpsimd.collective_compute(
                    "AllReduce", OP.add,
                    replica_groups=[list(range(N_CORES))],
                    ins=[ssm_p[b][:, :]],
                    outs=[ssm_f[b][:, :]],
                )

            def dt_proj(b, dl_cur, dt):
                """dt_proj -> softplus (batched Exp then batched Ln)."""
                exs = []
                for c in range(NCH):
                    csl = slice(c * CH, (c + 1) * CH)
                    lr = stg.tile([128, CH], BF16, name="lr", tag="lr", bufs=1)
                    nc.scalar.dma_start(lr[:], ssm_f[b][0:R, csl])
                    ps = ppA.tile([128, CH], F32, name="psD", tag="psA")
                    nc.tensor.matmul(ps[:], dtw_t[dt][:], lr[:],
                                     start=True, stop=True)
                    ex = stg.tile([128, CH], BF16, name="ex", tag=f"ex{c % 2}",
                                  bufs=1)
                    nc.scalar.activation(ex[:], ps[:], AF.Exp,
                                         bias=db_t[dt][:, 0:1])
                    exs.append((ex, csl))
                for ex, csl in exs:
                    nc.scalar.activation(dl_cur[dt][:, csl], ex[:], AF.Ln,
                                         bias=1.0)

            def prep_du(b, dl_cur, eng="vector"):
                for dt in range(MD):
                    getattr(nc, eng).tensor_mul(du_t[dt][:], dl_cur[dt][:],
                                                u_t[dt][:])

            def bcast_n(b, n):
                bt = bc.tile([128, L], BF16, name="Bt", tag="Bt", bufs=6)
                nc.sync.dma_start(
                    bt[:], ssm_f[b][R + n:R + n + 1, :].to_broadcast((128, L)))
                ct = bc.tile([128, L], BF16, name="Ct", tag="Ct", bufs=5)
                nc.sync.dma_start(
                    ct[:],
                    ssm_f[b][R + N + n:R + N + n + 1, :].to_broadcast((128, L)))
                return bt, ct

            def gating(b, y_cur, dts=None):
                boff = b * L
                for dt in (dts if dts is not None else range(MD)):
                    sg = wk.tile([128, L], BF16, name="sg", tag="hc", bufs=2)
                    nc.scalar.dma_start(
                        sg[:], sgd[dt * 128:(dt + 1) * 128, boff:boff + L])
                    nc.vector.tensor_mul(y_cur[dt][:], y_cur[dt][:], sg[:])

            def load_wo(mhalf):
                wo_k = []
                for kk in range(MD):
                    wt = wp.tile([128, H // 2], BF16, name=f"wo{kk}",
                                 tag=f"wk{kk}")
                    nc.sync.dma_start(
                        wt[:], woT[kk * 128:(kk + 1) * 128,
                                   mhalf * (H // 2):(mhalf + 1) * (H // 2)])
                    wo_k.append(wt)
                return wo_k

            def out_proj_m(b, m, wo_k, yo_cur, alt=False):
                boff = b * L
                ml = m % (KH // 2)
                wsl = slice(ml * 128, (ml + 1) * 128)
                osl = slice(m * 128, (m + 1) * 128)
                for hh in range(2):                   # L halves, 2-bank psum
                    hsl = slice(hh * (L // 2), (hh + 1) * (L // 2))
                    if alt and (m + hh) % 2:
                        ps = ppY.tile([128, L // 2], F32, name="psOy",
                                      tag="psY")
                    else:
                        ps = ppO.tile([128, L // 2], F32, name="psO",
                                      tag="psO")
                    for kk in range(MD):
                        for c in range(2):
                            csl = slice(hh * (L // 2) + c * CH,
                                        hh * (L // 2) + (c + 1) * CH)
                            psl = slice(c * CH, (c + 1) * CH)
                            nc.tensor.matmul(ps[:, psl], wo_k[kk][:, wsl],
                                             yo_cur[kk][:, csl],
                                             start=(kk == 0),
                                             stop=(kk == MD - 1))
                    so = stg.tile([128, L // 2], BF16, name="stO", tag="stO",
                                  bufs=1)
                    nc.scalar.copy(so[:], ps[:])
                    nc.scalar.dma_start(
                        out_part[osl, boff + hsl.start:boff + hsl.stop], so[:])

            # ---- scan group: one (n-block, dt) — 4 scans + PE y-accum ----
            def issue_dA_grp(dt, dl_cur, nb):
                tiles = []
                for j in range(4):
                    n = nb * 4 + j
                    dA = wk.tile([128, L], BF16, name="dA", tag="dA", bufs=4)
                    nc.scalar.activation(dA[:], dl_cur[dt][:], AF.Exp,
                                         scale=A_t[dt][:, n:n + 1])
                    tiles.append(dA)
                return tiles

            def scan_group(y_cur, dt, bts, cts, dAs):
                """4 scans of one dt over an n-block; y accumulated on PE
                into a [128,L] psum via identity matmuls."""
                psY = ppY.tile([128, L], F32, name="psY", tag="psY")
                for c in range(NCH):
                    csl = slice(c * CH, (c + 1) * CH)
                    nc.tensor.matmul(psY[:, csl], I_t[:], y_cur[dt][:, csl],
                                     start=True, stop=False,
                                     skip_group_check=True)
                for j in range(4):
                    dbu = wk.tile([128, L], BF16, name="dBu", tag="dBu",
                                  bufs=1)
                    nc.vector.tensor_mul(dbu[:], du_t[dt][:], bts[j][:])
                    h = wk.tile([128, L], BF16, name="h", tag="h", bufs=1)
                    nc.vector.tensor_tensor_scan(h[:], dAs[j][:], dbu[:],
                                                 0.0, op0=OP.mult, op1=OP.add)
                    hc = wk.tile([128, L], BF16, name="hc", tag="hc", bufs=2)
                    nc.vector.tensor_mul(hc[:], h[:], cts[j][:])
                    for c in range(NCH):
                        csl = slice(c * CH, (c + 1) * CH)
                        nc.tensor.matmul(psY[:, csl], I_t[:], hc[:, csl],
                                         start=False,
                                         stop=(j == 3 and c == NCH - 1),
                                         skip_group_check=True)
                nc.scalar.copy(y_cur[dt][:], psY[:])

            def issue_block0(b):
                bts, cts = [], []
                for j in range(4):
                    t1, t2 = bcast_n(b, j)
                    bts.append(t1)
                    cts.append(t2)
                return bts, cts

            # ================= schedule =================
            # ---- head: batch 0 x-pass, chunk-pipelined, chunked AR ----
            w_x = load_win(0)
            x0 = [tagA(dt, f"x0_{dt}") for dt in range(MD)]
            state["x"][0] = x0
            for c in range(NCH):
                in_proj_chunk(0, c, w_x, 0, x0)
                conv_pe(0, chunks=[c])
                x_proj(0, [c])
            all_reduce(0)
            pre0_b0 = issue_block0(0)
            dl0 = [tagB(dt, f"dl0_{dt}") for dt in range(MD)]

            def prep_b0_dt(dt):
                dt_proj(0, dl0, dt)
                nc.vector.tensor_mul(du_t[dt][:], dl0[dt][:], u_t[dt][:])
                nc.scalar.activation(y_t[dt][:], u_t[dt][:], AF.Copy,
                                     scale=D_t[dt][:, 0:1])

            prep_b0_dt(0)
            if cfg.get("DEBUG"):
                for dt in range(MD):
                    rs = slice(dt * 128, (dt + 1) * 128)
                    nc.sync.dma_start(dbg["dbg_x"][rs, :], x0[dt][:])
                    nc.sync.dma_start(dbg["dbg_u"][rs, :], u_t[dt][:])

            # ---- scan window 0 (batch 0): n-blocks of 4, dt-inner ----
            x1 = [tagA(dt, f"x1_{dt}") for dt in range(MD)]
            state["x"][1] = x1
            dl1 = None
            w_g = None

            def window(b, y_cur, dl_cur, interleave, pre0,
                       tail_prefetch=None):
                bts = [pre0[0], None]
                cts = [pre0[1], None]
                nxt_dA = issue_dA_grp(0, dl_cur, 0)
                for nb in range(4):
                    if nb + 1 < 4:
                        bts[1], cts[1] = [], []
                        for j in range(4):
                            t1, t2 = bcast_n(b, (nb + 1) * 4 + j)
                            bts[1].append(t1)
                            cts[1].append(t2)
                    if nb == 3 and tail_prefetch is not None:
                        tail_prefetch()
                    for dt in range(MD):
                        g = nb * 4 + dt
                        cur_dA = nxt_dA
                        interleave(g)
                        if g + 1 < 16:
                            nxt_dA = issue_dA_grp((dt + 1) % MD, dl_cur,
                                                  nb + (dt + 1) // MD)
                        scan_group(y_cur, dt, bts[0], cts[0], cur_dA)
                    bts[0], cts[0] = bts[1], cts[1]



            def interleave0(g):
                nonlocal dl1, w_g
                if 0 <= g <= 2:                    # deferred b0 prep
                    prep_b0_dt(g + 1)
                if 1 <= g <= 4:                    # in_proj-x (b1)
                    in_proj_chunk(1, g - 1, w_x, 0, x1)
                if 2 <= g <= 5:                    # conv (b1)
                    conv_pe(1, chunks=[g - 2])
                if g == 5:                         # x_proj (b1)
                    x_proj(1)
                    w_g = load_win(1)
                if g == 6:
                    all_reduce(1)
                if 7 <= g <= 10:                   # gate pass (b0)
                    in_proj_chunk(0, g - 7, w_g, 1, None)
                if 11 <= g <= 14:                  # dt_proj (b1)
                    if dl1 is None:
                        dl1 = [tagA(dt, f"dl1_{dt}") for dt in range(MD)]
                    dt_proj(1, dl1, g - 11)
                if g == 15:                        # y1 init (Act) early
                    for dt in range(MD):
                        nc.scalar.activation(y1h[dt][:], u_t[dt][:], AF.Copy,
                                             scale=D_t[dt][:, 0:1])

            y1h = [tagB(dt, f"y1_{dt}") for dt in range(MD)]
            pre_hold = {}

            def tail_pf():
                pre_hold["b1"] = issue_block0(1)

            window(0, y_t, dl0, interleave0, pre0_b0, tail_pf)

            if cfg.get("DEBUG"):
                for dt in range(MD):
                    rs = slice(dt * 128, (dt + 1) * 128)
                    nc.sync.dma_start(dbg["dbg_dl"][rs, :], dl0[dt][:])
                    nc.sync.dma_start(dbg["dbg_du"][rs, :], du_t[dt][:])
                    nc.sync.dma_start(dbg["dbg_y"][rs, :], y_t[dt][:])

            yo0 = y_t

            # ---- scan window 1 (batch 1) ----
            y1 = y1h
            prep_du(1, dl1, eng="vector")

            wo_state = {"wo": None}

            def interleave1(g):
                if g == 0:
                    gating(0, y_t)  # y0 *= silu(gate0), feeds out_proj at g>=5
                if 13 <= g <= 15:                  # gate y1 dts as they final
                    gating(1, y1, dts=[g - 13])
                if 1 <= g <= 4:                    # gate pass (b1)
                    in_proj_chunk(1, g - 1, w_g, 1, None)
                if g == 4:
                    wo_state["wo"] = load_wo(0)
                if 5 <= g <= 12:                   # out_proj(0) m=0..15
                    m = 2 * (g - 5)
                    if m == KH // 2:
                        wo_state["wo"] = load_wo(1)
                    out_proj_m(0, m, wo_state["wo"], yo0)
                    out_proj_m(0, m + 1, wo_state["wo"], yo0)

            window(1, y1, dl1, interleave1, pre_hold["b1"])

            # ---- tail: gate last dt + out_proj(1) ----
            gating(1, y1, dts=[3])
            wo_k = load_wo(0)
            for m in range(KH):
                if m == KH // 2:
                    wo_k = load_wo(1)
                out_proj_m(1, m, wo_k, y1, alt=True)

    _split_sync_waits(nc)
    return nc


def make_in_maps(cfg, hidden_states, in_proj_w, conv_w, conv_b, x_proj_w,
                 dt_proj_w, dt_proj_b, A_log, D_param, out_proj_w):
    import ml_dtypes
    BF = ml_dtypes.bfloat16
    H, IL, N, R, B, L = cfg["H"], cfg["IL"], cfg["N"], cfg["R"], cfg["B"], cfg["L"]
    MD = IL // 128
    KCONV = 4
    BL = B * L
    I_full = IL * N_CORES
    c = np.ascontiguousarray
    hs = np.asarray(hidden_states, np.float32)
    hsT = c(hs.reshape(BL, H).T.astype(BF))
    A_full = -np.exp(np.asarray(A_log, np.float32))
    in_proj_w = np.asarray(in_proj_w, np.float32)
    conv_w_f = np.asarray(conv_w, np.float32)
    in_maps = []
    for ci in range(N_CORES):
        sl = slice(ci * IL, (ci + 1) * IL)
        gsl = slice(I_full + ci * IL, I_full + (ci + 1) * IL)
        wxT = in_proj_w[sl, :].T
        wgT = in_proj_w[gsl, :].T
        convd = np.zeros((128, MD * KCONV * 128), np.float32)
        for dt in range(MD):
            for s in range(KCONV):
                w = conv_w_f[ci * IL + dt * 128:ci * IL + (dt + 1) * 128, 0,
                             KCONV - 1 - s]
                off = (dt * KCONV + s) * 128
                convd[:, off:off + 128][np.arange(128), np.arange(128)] = w
        in_maps.append({
            "hsT": hsT,
            "ident": np.eye(128, dtype=np.float32).astype(BF),
            "winT": c(np.concatenate([wxT, wgT], axis=1).astype(BF)),
            "convd": c(convd.astype(BF)),
            "convb": c(np.asarray(conv_b, np.float32)[sl].reshape(IL, 1)),
            "xwT": c(np.asarray(x_proj_w, np.float32)[:, sl].T.astype(BF)),
            "dtwT": c(np.asarray(dt_proj_w, np.float32)[sl, :].T.astype(BF)),
            "dtb": c(np.asarray(dt_proj_b, np.float32)[sl].reshape(IL, 1)),
            "Amat": c(A_full[sl, :]),
            "Dp": c(np.asarray(D_param, np.float32)[sl].reshape(IL, 1)),
            "woT": c(np.asarray(out_proj_w, np.float32)[:, sl].T.astype(BF)),
        })
    return in_maps


_PROG_CACHE = {}


def run(cfg, inputs, **run_kwargs):
    key = tuple(sorted((k, str(v)) for k, v in cfg.items()))
    if key not in _PROG_CACHE:
        _PROG_CACHE[key] = build_program(cfg)
    nc = _PROG_CACHE[key]
    in_maps = make_in_maps(cfg, **inputs)
    res = run_bass_kernel_spmd(nc, in_maps, list(range(N_CORES)), **run_kwargs)
    H, B, L = cfg["H"], cfg["B"], cfg["L"]
    out = np.zeros((H, B * L), np.float64)
    for ci in range(N_CORES):
        out += np.asarray(res.results[ci]["out_part"], np.float64)
    full = out.astype(np.float32).T.reshape(B, L, H)
    return full, res


def kernel(**inputs):
    out, _ = run(CFG_FULL, inputs)
    return out
